# revision 8
# baseline (speedup 1.0000x reference)
"""nn_DecoderLayer (MLA attention + MoE routing) on 8 TRN2 NeuronCores.

Strategy:
  NEFF1 (attention): head-parallel — core c computes heads {2c, 2c+1}:
    replicated q_a/kv_a down-projections (feature-major, fp32r matmuls),
    per-head q_b/kv_b + RoPE (rotate-half folded into host-augmented
    weights), causal scoresT [k,q] layout, exp softmax without max
    subtraction (|scores| ~ 1.5), AV accumulate, partial o-projection.
    Host sums the 8 o-partials (expert-parallel style combine), adds
    residual, computes rmsnorm + router + top-4 routing in numpy.
  NEFF2 (MoE): expert-parallel — core c owns experts {2c, 2c+1}: gathered
    per-expert token batches (capacity CAP) through gate/up/silu/down with
    the combine weight folded into the activation; shared expert is
    token-parallel (core c handles tokens [256c, 256c+256)).
  Host scatters expert outputs back, adds shared + residual.

All matmuls fp32r (measured ~1.5e-4 rel err on HW at full PE rate).
"""
import math
import numpy as np

import concourse.bacc as bacc
import concourse.mybir as mybir
import concourse.tile as tile
from concourse import bass_utils
from concourse.bass import ts

# problem dims
S, H = 2048, 2048
NH, NOPE, ROPE, DV = 16, 128, 64, 128
DQK = NOPE + ROPE                  # 192
QR, KVR = 768, 512
E, KTOP, MI = 16, 4, 1024
SCALE = 2.5
EPS = 1e-6
ROPE_BASE = 10000.0

NC = 8                              # cores
HPC = NH // NC                      # heads/core = 2
EPC = E // NC                       # experts/core = 2
CAP = 640                           # per-expert token capacity (max seen ~547)
P = 128
NSTRIP = S // 512                   # 4 strips of 512 tokens

F32 = mybir.dt.float32
F32R = mybir.dt.float32r

Exp = mybir.ActivationFunctionType.Exp
Sqrt = mybir.ActivationFunctionType.Sqrt
Square = mybir.ActivationFunctionType.Square
Silu = mybir.ActivationFunctionType.Silu
Identity = mybir.ActivationFunctionType.Identity

_cache = {}


# ---------------------------------------------------------------- NEFF 1
def build_neff1():
    nc = bacc.Bacc("TRN2", num_devices=NC, debug=False)
    def inp(name, shape):
        return nc.dram_tensor(name, list(shape), F32, kind="ExternalInput").ap()

    xTp = inp("xTp", (NSTRIP, P, 16, 512))        # xTp[s,p,c,j] = x[512s+j, 128c+p]
    qaw = inp("qaw", (P, 6, 16, P))                # lhsT: [p, m_tile, h_chunk, j]
    kvaw = inp("kvaw", (P, 4, 16, P))
    qbw = inp("qbw", (P, 6, 512))                  # lhsT: [p, qr_chunk, m] (2 heads x 256)
    kvbw = inp("kvbw", (P, 4, 768))                # lhsT: [p, kvr_chunk, m] (2 heads x 384)
    ow = inp("ow", (P, HPC, H))                    # rhs: [p, c_chunk, h]
    cosT = inp("cosT", (ROPE, S))
    sinT = inp("sinT", (ROPE, S))
    masks = inp("masks", (P, 4, 512))              # tri01 keep-mask per diag offset
    o_part = nc.dram_tensor("o_part", [S, H], F32, kind="ExternalOutput").ap()

    with tile.TileContext(nc) as tc:
        with tc.tile_pool(name="const", bufs=1) as cpool, \
             tc.tile_pool(name="dram", bufs=1, space="DRAM") as dr:

            ones_f = cpool.tile([P, 1], F32)
            nc.vector.memset(ones_f, 1.0)
            ones_r = cpool.tile([P, 1], F32R)
            nc.scalar.copy(ones_r, ones_f)
            eps_t = cpool.tile([P, 1], F32)
            nc.vector.memset(eps_t, EPS)

            # DRAM scratch
            qnT_d = dr.tile([6, P, S], F32R)
            kvnT_d = dr.tile([4, P, S], F32R)
            sig_d = dr.tile([NSTRIP, 2, 1, 512], F32)       # sigma rows bounce
            rcp_d = dr.tile([NSTRIP, HPC, 1, 512], F32)     # softmax denom bounce
            qfT_d = dr.tile([HPC, 2, P, S], F32R)           # chunk0 nope, chunk1[0:64] rope
            kfT_d = dr.tile([HPC, 2, P, S], F32R)
            v_d = dr.tile([HPC, 16, P, DV], F32R)           # token-major v chunks

            # ---------------- Pass A: x -> qnT / kvnT (feature-major)
            with tc.tile_pool(name="pa", bufs=1) as pa, \
                 tc.tile_pool(name="pa2", bufs=2) as pa2, \
                 tc.tile_pool(name="pa1", bufs=1) as pa1, \
                 tc.tile_pool(name="pa3", bufs=3) as pa3, \
                 tc.tile_pool(name="psA", bufs=2, space="PSUM") as psA:
                qaw_sb = pa.tile([P, 6, 16, P], F32R)
                nc.sync.dma_start(out=qaw_sb, in_=qaw.bitcast(F32R))
                kvaw_sb = pa.tile([P, 4, 16, P], F32R)
                nc.sync.dma_start(out=kvaw_sb, in_=kvaw.bitcast(F32R))

                for s in range(NSTRIP):
                    xs = pa2.tile([P, 16, 512], F32R, tag="xs")
                    nc.sync.dma_start(out=xs, in_=xTp[s].bitcast(F32R))

                    # sum(x^2) over H per token -> row [1, 512]
                    sx_ps = psA.tile([1, 512], F32, tag="row")
                    for c in range(16):
                        sq = pa3.tile([P, 512], F32R, tag="sq")
                        nc.scalar.activation(out=sq, in_=xs[:, c, :].bitcast(F32),
                                             func=Square)
                        nc.tensor.matmul(sx_ps, ones_r, sq,
                                         start=(c == 0), stop=(c == 15))
                    # a = sx/2048 + eps
                    a_row = pa2.tile([1, 512], F32, tag="arow")
                    nc.scalar.activation(out=a_row, in_=sx_ps, func=Identity,
                                         bias=eps_t[0:1, :], scale=1.0 / H)

                    for kind in range(2):  # 0: q (6 m-tiles), 1: kv (4 m-tiles)
                        mt = 6 if kind == 0 else 4
                        wsb = qaw_sb if kind == 0 else kvaw_sb
                        fdim = QR if kind == 0 else KVR
                        raw = pa1.tile([P, mt, 512], F32R, tag=f"raw{kind}")
                        msq_ps = psA.tile([1, 512], F32, tag="row")
                        for m in range(mt):
                            mm_ps = psA.tile([P, 512], F32, tag="mm")
                            for c in range(16):
                                nc.tensor.matmul(mm_ps, wsb[:, m, c, :],
                                                 xs[:, c, :],
                                                 start=(c == 0), stop=(c == 15))
                            nc.scalar.copy(raw[:, m, :], mm_ps)
                            sq = pa3.tile([P, 512], F32R, tag="sq")
                            nc.scalar.activation(out=sq, in_=mm_ps, func=Square)
                            nc.tensor.matmul(msq_ps, ones_r, sq,
                                             start=(m == 0), stop=(m == mt - 1))
                        # sigma = 1/sqrt(sum_v2/fdim + eps*a)
                        msq_row = pa2.tile([1, 512], F32, tag="msqr")
                        nc.scalar.mul(msq_row, msq_ps, 1.0 / fdim)
                        sig = pa2.tile([1, 512], F32, tag="sig")
                        nc.vector.scalar_tensor_tensor(
                            out=sig, in0=a_row, scalar=float(EPS), in1=msq_row,
                            op0=mybir.AluOpType.mult, op1=mybir.AluOpType.add)
                        nc.scalar.activation(out=sig, in_=sig, func=Sqrt)
                        nc.vector.reciprocal(sig, sig)
                        nc.sync.dma_start(out=sig_d[s, kind], in_=sig)
                        sig_bc = pa2.tile([P, 512], F32, tag=f"sbc{kind}")
                        nc.sync.dma_start(out=sig_bc,
                                          in_=sig_d[s, kind].broadcast_to((P, 512)))
                        for m in range(mt):
                            nc.vector.tensor_mul(raw[:, m, :],
                                                 raw[:, m, :].bitcast(F32), sig_bc)
                        dst = qnT_d if kind == 0 else kvnT_d
                        nc.sync.dma_start(
                            out=dst[:, :, 512 * s:512 * (s + 1)].rearrange(
                                "c p j -> p c j"),
                            in_=raw)

            # ---------------- Pass B: q_b / kv_b + RoPE -> qfT_d, kfT_d, v_d
            with tc.tile_pool(name="pb", bufs=1) as pb, \
                 tc.tile_pool(name="pb2", bufs=2) as pb2, \
                 tc.tile_pool(name="pb3", bufs=3) as pb3, \
                 tc.tile_pool(name="psB", bufs=2, space="PSUM") as psB:
                qbw_sb = pb.tile([P, 6, 512], F32R)
                nc.sync.dma_start(out=qbw_sb, in_=qbw.bitcast(F32R))
                kvbw_sb = pb.tile([P, 4, 768], F32R)
                nc.sync.dma_start(out=kvbw_sb, in_=kvbw.bitcast(F32R))
                cos_sb = pb.tile([ROPE, S], F32)
                nc.sync.dma_start(out=cos_sb, in_=cosT)
                sin_sb = pb.tile([ROPE, S], F32)
                nc.sync.dma_start(out=sin_sb, in_=sinT)

                for s in range(NSTRIP):
                    sl = slice(512 * s, 512 * (s + 1))
                    qn = []
                    for c in range(6):
                        t = pb3.tile([P, 512], F32R, tag=f"qn{c}")
                        nc.sync.dma_start(out=t, in_=qnT_d[c, :, sl])
                        qn.append(t)
                    kvn = []
                    for c in range(4):
                        t = pb3.tile([P, 512], F32R, tag=f"kvn{c}")
                        nc.sync.dma_start(out=t, in_=kvnT_d[c, :, sl])
                        kvn.append(t)

                    for hi in range(HPC):
                        for side in range(2):  # 0: q, 1: k/v
                            if side == 0:
                                wsb, chunks, base, nmt = qbw_sb, qn, 256 * hi, 6
                                dstf = qfT_d
                            else:
                                wsb, chunks, base, nmt = kvbw_sb, kvn, 384 * hi, 4
                                dstf = kfT_d
                            ps_n = psB.tile([P, 512], F32, tag="bn")
                            ps_ro = psB.tile([64, 512], F32, tag="bro")
                            ps_rt = psB.tile([64, 512], F32, tag="brt")
                            for c in range(nmt):
                                st, sp = (c == 0), (c == nmt - 1)
                                nc.tensor.matmul(ps_n, wsb[:, c, base:base + 128],
                                                 chunks[c], start=st, stop=sp)
                                nc.tensor.matmul(ps_ro,
                                                 wsb[:, c, base + 128:base + 192],
                                                 chunks[c], start=st, stop=sp)
                                nc.tensor.matmul(ps_rt,
                                                 wsb[:, c, base + 192:base + 256],
                                                 chunks[c], start=st, stop=sp)
                            stage_n = pb2.tile([P, 512], F32R, tag="stn")
                            nc.scalar.copy(stage_n, ps_n)
                            nc.sync.dma_start(out=dstf[hi, 0, :, sl], in_=stage_n)
                            t1 = pb3.tile([64, 512], F32, tag="t1")
                            nc.vector.tensor_mul(t1, ps_ro, cos_sb[:, sl])
                            stage_r = pb2.tile([64, 512], F32R, tag="str")
                            nc.vector.tensor_mul(stage_r, ps_rt, sin_sb[:, sl])
                            nc.vector.tensor_add(stage_r, stage_r.bitcast(F32), t1)
                            nc.sync.dma_start(out=dstf[hi, 1, 0:64, sl], in_=stage_r)
                            if side == 1:
                                for t in range(4):
                                    ps_v = psB.tile([P, P], F32, tag="bv")
                                    for c in range(4):
                                        nc.tensor.matmul(
                                            ps_v, kvn[c][:, ts(t, P)],
                                            kvbw_sb[:, c, base + 256:base + 384],
                                            start=(c == 0), stop=(c == 3))
                                    stage_v = pb2.tile([P, P], F32R, tag="stv")
                                    nc.scalar.copy(stage_v, ps_v)
                                    nc.sync.dma_start(out=v_d[hi, 4 * s + t],
                                                      in_=stage_v)

            # ---------------- Pass C: attention + o-projection
            with tc.tile_pool(name="pc", bufs=1) as pc, \
                 tc.tile_pool(name="pc2", bufs=2) as pc2, \
                 tc.tile_pool(name="pc3", bufs=3) as pc3, \
                 tc.tile_pool(name="psC", bufs=2, space="PSUM") as psC, \
                 tc.tile_pool(name="psD", bufs=1, space="PSUM") as psD:
                mask_sb = pc.tile([P, 4, 512], F32)
                nc.sync.dma_start(out=mask_sb, in_=masks)
                ow_sb = pc.tile([P, HPC, H], F32R)
                nc.sync.dma_start(out=ow_sb, in_=ow.bitcast(F32R))

                for s in range(NSTRIP):
                    sl = slice(512 * s, 512 * (s + 1))
                    ctx_sb = pc2.tile([P, HPC, 512], F32R, tag="ctx")
                    for hi in range(HPC):
                        qf_n = pc2.tile([P, 512], F32R, tag="qfn")
                        nc.sync.dma_start(out=qf_n, in_=qfT_d[hi, 0, :, sl])
                        qf_r = pc2.tile([64, 512], F32R, tag="qfr")
                        nc.sync.dma_start(out=qf_r, in_=qfT_d[hi, 1, 0:64, sl])
                        ps_ctx = psD.tile([P, 512], F32, tag="ctx")
                        ps_den = psD.tile([1, 512], F32, tag="den")
                        nkc = 4 * s + 4
                        for ks in range(s + 1):
                            kf_n = pc3.tile([P, 512], F32R, tag="kfn")
                            nc.sync.dma_start(
                                out=kf_n, in_=kfT_d[hi, 0, :, 512 * ks:512 * (ks + 1)])
                            kf_r = pc3.tile([64, 512], F32R, tag="kfr")
                            nc.sync.dma_start(
                                out=kf_r,
                                in_=kfT_d[hi, 1, 0:64, 512 * ks:512 * (ks + 1)])
                            v_t = pc3.tile([P, 4, DV], F32R, tag="vt")
                            nc.sync.dma_start(
                                out=v_t, in_=v_d[hi, 4 * ks:4 * ks + 4].rearrange(
                                    "c p d -> p c d"))
                            for kq in range(4):
                                kc = 4 * ks + kq
                                ps_sc = psC.tile([P, 512], F32, tag="sc")
                                nc.tensor.matmul(ps_sc, kf_n[:, ts(kq, P)], qf_n,
                                                 start=True, stop=False)
                                nc.tensor.matmul(ps_sc, kf_r[:, ts(kq, P)], qf_r,
                                                 start=False, stop=True)
                                att = pc3.tile([P, 512], F32R, tag="att")
                                nc.scalar.activation(out=att, in_=ps_sc, func=Exp)
                                if kc >= 4 * s:
                                    nc.vector.tensor_mul(att, att.bitcast(F32),
                                                         mask_sb[:, kc - 4 * s, :])
                                nc.tensor.matmul(ps_den, ones_r, att,
                                                 start=(kc == 0), stop=(kc == nkc - 1))
                                nc.tensor.matmul(ps_ctx, v_t[:, kq, :], att,
                                                 start=(kc == 0), stop=(kc == nkc - 1))
                        den = pc2.tile([1, 512], F32, tag="den")
                        nc.scalar.copy(den, ps_den)
                        rcp = pc2.tile([1, 512], F32, tag="rcp")
                        nc.vector.reciprocal(rcp, den)
                        nc.sync.dma_start(out=rcp_d[s, hi], in_=rcp)
                        rcp_bc = pc2.tile([P, 512], F32, tag="rbc")
                        nc.sync.dma_start(out=rcp_bc,
                                          in_=rcp_d[s, hi].broadcast_to((P, 512)))
                        nc.vector.tensor_mul(ctx_sb[:, hi, :], ps_ctx, rcp_bc)
                    # o-projection for this strip (token-major out)
                    for t in range(4):
                        o_sb = pc2.tile([P, H], F32, tag="osb")
                        for hs in range(4):
                            ps_o = psC.tile([P, 512], F32, tag="o")
                            for cc in range(HPC):
                                nc.tensor.matmul(ps_o, ctx_sb[:, cc, ts(t, P)],
                                                 ow_sb[:, cc, ts(hs, 512)],
                                                 start=(cc == 0), stop=(cc == HPC - 1))
                            nc.scalar.copy(o_sb[:, ts(hs, 512)], ps_o)
                        nc.sync.dma_start(
                            out=o_part.rearrange("(T p) h -> p T h", p=P)[:, 4 * s + t, :],
                            in_=o_sb)
    nc.compile()
    return nc


# ---------------------------------------------------------------- NEFF 2
def build_neff2():
    nc = bacc.Bacc("TRN2", num_devices=NC, debug=False)
    def inp(name, shape):
        return nc.dram_tensor(name, list(shape), F32, kind="ExternalInput").ap()

    xe = inp("xe", (EPC, 16, P, CAP))        # gathered expert tokens, feature-major
    gw = inp("gw", (EPC, 8, P, 2048))        # gate lhsT prepack
    uw = inp("uw", (EPC, 8, P, 2048))
    dw = inp("dw", (EPC, 8, P, 2048))        # down rhs-layout [m_chunk, p, h]
    wrow = inp("wrow", (EPC, 1, CAP))        # combine weights (row layout)
    h2t = inp("h2t", (16, P, 256))           # my 256 tokens, feature-major
    sgw = inp("sgw", (8, P, 2048))
    suw = inp("suw", (8, P, 2048))
    sdw = inp("sdw", (8, P, 2048))
    yrT = nc.dram_tensor("yrT", [EPC, 16, P, CAP], F32, kind="ExternalOutput").ap()
    yshT = nc.dram_tensor("yshT", [16, P, 256], F32, kind="ExternalOutput").ap()

    NS = CAP // 320  # 2 strips of 320

    with tile.TileContext(nc) as tc:
        with tc.tile_pool(name="p1", bufs=1) as p1, \
             tc.tile_pool(name="pw", bufs=3) as pw, \
             tc.tile_pool(name="pact", bufs=2) as pact, \
             tc.tile_pool(name="py", bufs=2) as py, \
             tc.tile_pool(name="ps", bufs=2, space="PSUM") as ps:

            for i in range(EPC):
                xe_sb = p1.tile([P, 16, CAP], F32R, tag="xe")
                nc.sync.dma_start(out=xe_sb, in_=xe[i].rearrange(
                    "hc p t -> p hc t").bitcast(F32R))
                w_bc = pact.tile([P, CAP], F32, tag="wbc")
                nc.sync.dma_start(out=w_bc, in_=wrow[i].broadcast_to((P, CAP)))
                act = pact.tile([P, 8, CAP], F32R, tag="act")
                for t in range(8):
                    g_w = pw.tile([P, 16, P], F32R, tag="gw")
                    nc.sync.dma_start(out=g_w, in_=gw[i, t].rearrange(
                        "p (hc j) -> p hc j", j=P).bitcast(F32R))
                    u_w = pw.tile([P, 16, P], F32R, tag="uw")
                    nc.sync.dma_start(out=u_w, in_=uw[i, t].rearrange(
                        "p (hc j) -> p hc j", j=P).bitcast(F32R))
                    for half in range(NS):
                        hsl = slice(320 * half, 320 * (half + 1))
                        ps_g = ps.tile([P, 320], F32, tag="g")
                        ps_u = ps.tile([P, 320], F32, tag="u")
                        for hc in range(16):
                            st, sp = (hc == 0), (hc == 15)
                            nc.tensor.matmul(ps_g, g_w[:, hc, :], xe_sb[:, hc, hsl],
                                             start=st, stop=sp)
                            nc.tensor.matmul(ps_u, u_w[:, hc, :], xe_sb[:, hc, hsl],
                                             start=st, stop=sp)
                        sil = pw.tile([P, 320], F32, tag="sil")
                        nc.scalar.activation(out=sil, in_=ps_g, func=Silu)
                        nc.vector.tensor_mul(act[:, t, hsl], sil, ps_u)
                # fold combine weight into act
                for t in range(8):
                    nc.vector.tensor_mul(act[:, t, :], act[:, t, :].bitcast(F32), w_bc)
                # down: feature-major out yT [h_tile, tok]
                for ht in range(16):
                    y_sb = py.tile([P, CAP], F32, tag="ysb")
                    for half in range(NS):
                        hsl = slice(320 * half, 320 * (half + 1))
                        ps_y = ps.tile([P, 320], F32, tag="y")
                        for mc in range(8):
                            d_w = pw.tile([P, P], F32R, tag="dw")
                            nc.sync.dma_start(
                                out=d_w, in_=dw[i, mc, :, ts(ht, P)].bitcast(F32R))
                            nc.tensor.matmul(ps_y, d_w, act[:, mc, hsl],
                                             start=(mc == 0), stop=(mc == 7))
                        nc.scalar.copy(y_sb[:, hsl], ps_y)
                    nc.sync.dma_start(out=yrT[i, ht], in_=y_sb)

            # ---------------- shared expert (my 256 tokens)
            h2_sb = p1.tile([P, 16, 256], F32R, tag="h2")
            nc.sync.dma_start(out=h2_sb, in_=h2t.rearrange(
                "hc p t -> p hc t").bitcast(F32R))
            acts = pact.tile([P, 8, 256], F32R, tag="acts")
            for t in range(8):
                g_w = pw.tile([P, 16, P], F32R, tag="gw")
                nc.sync.dma_start(out=g_w, in_=sgw[t].rearrange(
                    "p (hc j) -> p hc j", j=P).bitcast(F32R))
                u_w = pw.tile([P, 16, P], F32R, tag="uw")
                nc.sync.dma_start(out=u_w, in_=suw[t].rearrange(
                    "p (hc j) -> p hc j", j=P).bitcast(F32R))
                ps_g = ps.tile([P, 256], F32, tag="g")
                ps_u = ps.tile([P, 256], F32, tag="u")
                for hc in range(16):
                    st, sp = (hc == 0), (hc == 15)
                    nc.tensor.matmul(ps_g, g_w[:, hc, :], h2_sb[:, hc, :],
                                     start=st, stop=sp)
                    nc.tensor.matmul(ps_u, u_w[:, hc, :], h2_sb[:, hc, :],
                                     start=st, stop=sp)
                sil = pw.tile([P, 256], F32, tag="ssil")
                nc.scalar.activation(out=sil, in_=ps_g, func=Silu)
                nc.vector.tensor_mul(acts[:, t, :], sil, ps_u)
            for ht in range(16):
                ps_y = ps.tile([P, 256], F32, tag="y")
                for mc in range(8):
                    d_w = pw.tile([P, P], F32R, tag="dw")
                    nc.sync.dma_start(out=d_w, in_=sdw[mc, :, ts(ht, P)].bitcast(F32R))
                    nc.tensor.matmul(ps_y, d_w, acts[:, mc, :],
                                     start=(mc == 0), stop=(mc == 7))
                y_sb = py.tile([P, 256], F32, tag="sysb")
                nc.scalar.copy(y_sb, ps_y)
                nc.sync.dma_start(out=yshT[ht], in_=y_sb)
    nc.compile()
    return nc


# ---------------------------------------------------------------- host prep
def _rope_tables():
    inv = 1.0 / (ROPE_BASE ** (np.arange(0, ROPE, 2, dtype=np.float64) / ROPE))
    t = np.arange(S, dtype=np.float64)
    f = t[:, None] * inv[None, :]
    emb = np.concatenate([f, f], axis=-1)          # [S, 64]
    return (np.cos(emb).T.astype(np.float32).copy(),
            np.sin(emb).T.astype(np.float32).copy())


def _lhsT_prepack(wT, mtiles):
    """wT [K, M] -> [P, mtiles, K//P, P]: SBUF-image for resident lhsT tiles."""
    Kd, Md = wT.shape
    assert Md == mtiles * P and Kd % P == 0
    return np.ascontiguousarray(
        wT.reshape(Kd // P, P, mtiles, P).transpose(1, 2, 0, 3))


def _lhsT_prepack2(wT, mtiles):
    """wT [K, M] -> [mtiles, P, K]: per-m-tile contiguous DMA layout.

    Tile t, flattened [P, K] with per-partition layout (hc, j):
    A[t, p, 128*hc + j] = wT[128*hc + p, 128*t + j].
    """
    Kd, Md = wT.shape
    assert Md == mtiles * P and Kd % P == 0
    return np.ascontiguousarray(
        wT.reshape(Kd // P, P, mtiles, P).transpose(2, 1, 0, 3).reshape(
            mtiles, P, Kd))


def _neff1_inputs(x, w):
    ln1 = w["ln1_w"]
    xT = x.T.astype(np.float32)                                   # [H, S]
    xTp = np.ascontiguousarray(xT.reshape(16, P, NSTRIP, 512).transpose(2, 1, 0, 3))

    WqT = (w["q_a_w"] * ln1[None, :]).T.astype(np.float32)        # [H, QR]
    WkvT = (w["kv_a_w"] * ln1[None, :]).T.astype(np.float32)      # [H, KVR]
    qaw = _lhsT_prepack(WqT, 6)
    kvaw = _lhsT_prepack(WkvT, 4)

    qb = (w["q_b_w"] * w["q_a_ln"][None, :]).astype(np.float32)   # [NH*DQK, QR]
    kvb = (w["kv_b_w"] * w["kv_a_ln"][None, :]).astype(np.float32)  # [NH*320, KVR]
    sc = 1.0 / math.sqrt(DQK)
    cosT, sinT = _rope_tables()

    masks = np.zeros((P, 4, 512), np.float32)
    pp, jj = np.meshgrid(np.arange(P), np.arange(512), indexing="ij")
    for cl in range(4):
        masks[:, cl, :] = (P * cl + pp <= jj).astype(np.float32)

    per_core = []
    for c in range(NC):
        heads = [HPC * c + i for i in range(HPC)]
        # q_b augmented: per head rows [nope 128 | rope 64 | rot 64], scaled by sc
        qrows = []
        for h in heads:
            blk = qb[h * DQK:(h + 1) * DQK] * sc                   # [192, QR]
            nope, rope = blk[:NOPE], blk[NOPE:]
            rot = np.concatenate([-rope[32:], rope[:32]], axis=0)
            qrows.append(np.concatenate([nope, rope, rot], axis=0))  # [256, QR]
        qaug = np.concatenate(qrows, axis=0)                       # [512, QR]
        qbw = np.ascontiguousarray(qaug.T.reshape(6, P, 512).transpose(1, 0, 2))

        kvrows = []
        for h in heads:
            blk = kvb[h * 320:(h + 1) * 320]                       # [320, KVR]
            kn, kr, vv = blk[:NOPE], blk[NOPE:DQK], blk[DQK:]
            krot = np.concatenate([-kr[32:], kr[:32]], axis=0)
            kvrows.append(np.concatenate([kn, kr, krot, vv], axis=0))  # [384, KVR]
        kvaug = np.concatenate(kvrows, axis=0)                     # [768, KVR]
        kvbw = np.ascontiguousarray(kvaug.T.reshape(4, P, 768).transpose(1, 0, 2))

        ocols = np.concatenate([w["o_w"][:, h * DV:(h + 1) * DV] for h in heads],
                               axis=1)                             # [H, 256]
        owp = np.ascontiguousarray(ocols.T.reshape(HPC, P, H).transpose(1, 0, 2))

        per_core.append({
            "xTp": xTp, "qaw": qaw, "kvaw": kvaw,
            "qbw": qbw.astype(np.float32), "kvbw": kvbw.astype(np.float32),
            "ow": owp.astype(np.float32), "cosT": cosT, "sinT": sinT,
            "masks": masks,
        })
    return per_core


def _route(h2ln, w):
    """Top-4 routing in numpy (fp32, matches jax semantics for these gaps)."""
    logits = h2ln @ w["router_w"].T.astype(np.float32) + w["router_b"][None, :]
    probs = 1.0 / (1.0 + np.exp(-logits))
    order = np.argsort(-probs, axis=-1, kind="stable")[:, :KTOP]
    topv = np.take_along_axis(probs, order, axis=-1)
    wts = topv / (topv.sum(-1, keepdims=True) + 1e-9) * SCALE
    return order, wts


def _neff2_inputs(h2, h2ln, w, order, wts):
    """h2: rmsnorm w/o ln2 (expert input pre-ln2-fold); h2ln unused here."""
    ln2 = w["ln2_w"]
    idx_lists, wt_lists = [], []
    for e in range(E):
        tok, kk = np.where(order == e)
        idx_lists.append(tok)
        wt_lists.append(wts[tok, kk])

    h2T = np.ascontiguousarray(h2.T)                        # [H, S]
    per_core = []
    spill = []                                              # (expert, tok, wt) overflow
    sgw = _lhsT_prepack2((w["sg_w"] * ln2[None, :]).T.astype(np.float32), 8)
    suw = _lhsT_prepack2((w["su_w"] * ln2[None, :]).T.astype(np.float32), 8)
    sdw = np.ascontiguousarray(w["sd_w"].T.reshape(8, P, H)).astype(np.float32)

    for c in range(NC):
        xeb = np.zeros((EPC, 16, P, CAP), np.float32)
        gwb = np.zeros((EPC, 8, P, 2048), np.float32)
        uwb = np.zeros((EPC, 8, P, 2048), np.float32)
        dwb = np.zeros((EPC, 8, P, 2048), np.float32)
        wrow = np.zeros((EPC, 1, CAP), np.float32)
        for i in range(EPC):
            e = EPC * c + i
            tok, tw = idx_lists[e], wt_lists[e]
            if len(tok) > CAP:
                spill.append((e, tok[CAP:], tw[CAP:]))
                tok, tw = tok[:CAP], tw[:CAP]
            n = len(tok)
            xeb[i, :, :, :n] = h2T[:, tok].reshape(16, P, n)
            wrow[i, 0, :n] = tw
            gwb[i] = _lhsT_prepack2(
                (w["gate_w"][e] * ln2[None, :]).T.astype(np.float32), 8)
            uwb[i] = _lhsT_prepack2(
                (w["up_w"][e] * ln2[None, :]).T.astype(np.float32), 8)
            dwb[i] = w["down_w"][e].T.reshape(8, P, H).astype(np.float32)
        rows = slice(256 * c, 256 * (c + 1))
        h2tp = np.ascontiguousarray(h2T[:, rows].reshape(16, P, 256)).astype(np.float32)
        per_core.append({
            "xe": xeb, "gw": gwb, "uw": uwb, "dw": dwb, "wrow": wrow,
            "h2t": h2tp, "sgw": sgw, "suw": suw, "sdw": sdw,
        })
    return per_core, idx_lists, wt_lists, spill


def _expert_np(h2ln, idx, wt, w, e):
    """Numpy fallback for capacity-overflow tokens."""
    xg = h2ln[idx]
    g = xg @ w["gate_w"][e].T
    u = xg @ w["up_w"][e].T
    a = (g / (1 + np.exp(-g))) * u
    return (a @ w["down_w"][e].T) * wt[:, None]


# ---------------------------------------------------------------- kernel
def kernel(**inputs):
    w = {k: np.asarray(v, dtype=np.float32) for k, v in inputs.items()}
    x = w["x"][0]                                           # [S, H]

    if "nc1" not in _cache:
        _cache["nc1"] = build_neff1()
    nc1 = _cache["nc1"]
    in1 = _neff1_inputs(x, w)
    res1 = bass_utils.run_bass_kernel_spmd(nc1, in1, core_ids=list(range(NC)),
                                           trace=False)
    o_sum = np.zeros((S, H), np.float32)
    for c in range(NC):
        o_sum += res1.results[c]["o_part"]
    x2 = x + o_sum

    r2 = 1.0 / np.sqrt((x2 * x2).mean(-1, keepdims=True) + EPS)
    h2 = (x2 * r2).astype(np.float32)                       # rmsnorm w/o ln2
    h2ln = h2 * w["ln2_w"][None, :]
    order, wts = _route(h2ln, w)

    if "nc2" not in _cache:
        _cache["nc2"] = build_neff2()
    nc2 = _cache["nc2"]
    in2, idx_lists, wt_lists, spill = _neff2_inputs(h2, h2ln, w, order, wts)
    res2 = bass_utils.run_bass_kernel_spmd(nc2, in2, core_ids=list(range(NC)),
                                           trace=False)

    out = x2.copy()
    for c in range(NC):
        r = res2.results[c]
        for i in range(EPC):
            e = EPC * c + i
            tok = idx_lists[e][:CAP]
            ye = r["yrT"][i].reshape(H, CAP).T              # [CAP, H]
            out[tok] += ye[:len(tok)]
        out[256 * c:256 * (c + 1)] += r["yshT"].reshape(H, 256).T
    for e, tok, tw in spill:
        out[tok] += _expert_np(h2ln, tok, tw, w, e)
    return out.reshape(1, S, H).astype(np.float32)
